# revision 1
# baseline (speedup 1.0000x reference)
"""Trainium2 Bass kernel for the GraphicalBranch GNN message-passing problem.

Math being computed (verified equivalent to the reference):
  - Per-sample graphs are fully connected WITH self-loops over the nc2=28
    pair-nodes, so segment_sum(x[src], dst) == broadcast of the per-sample
    row-sum S[b] = sum_r x[b, r, :].
  - The final key-matching gather h[rows] commutes with the row-wise linear
    layer, so we only run the W_self matmul on the 10 gathered rows per
    sample instead of all 28:
        out[b*10+k] = relu(xg[b*10+k] @ W_self + (S[b] @ W_nbr) + b)
  - rows are computed on host from slicing_tensor/object_pairs (pure index
    arithmetic) exactly as the reference's LUT does.

Sharding: data-parallel over samples; each of the 8 cores gets 128 samples
(3584 x-rows, 1280 output rows). Weights replicated.

Per-core device program (all matmul operands bf16, f32 PSUM accumulate):
  1. Load xT (x-slab transposed, [512, 3584]) in 4 partition-tiles; VectorE
     reduces groups of 28 columns -> S^T [128, 4, 128] f32; cast to bf16.
  2. A = S @ W_nbr via 4 accumulating matmuls (lhsT = S^T tiles); add
     bias; cast A -> bf16.
  3. For each of 10 output tiles of 128 rows: 4 accumulating matmuls
     xg-tile @ W_self (lhsT = xgT slices) plus one expansion matmul
     E-tile @ A (E[r, s] = 1 iff s == r//10) which adds A[r//10] to row r
     inside the same PSUM accumulation group; ReLU via ScalarE; DMA out.
"""

import numpy as np
import ml_dtypes

# ---- problem constants (hardcoded; kernel.py must be self-contained) ----
B = 1024          # samples
NOBJ = 8          # objects per sample
NC2 = 28          # pair-nodes per sample
MAXR = 10         # relations per sample
D = 512           # feature dim
NCORES = 8
BL = B // NCORES          # 128 samples per core
RL = BL * NC2             # 3584 x-rows per core
ML = BL * MAXR            # 1280 output rows per core
KT = D // 128             # 4 contraction tiles
MT = ML // 128            # 10 output row tiles per core

BF16 = ml_dtypes.bfloat16

_compiled = None  # (nc, core_ids) cache: build/compile once per process


def _build_bass():
    import concourse.bacc as bacc
    import concourse.bass as bass
    import concourse.mybir as mybir
    from concourse import tile

    f32 = mybir.dt.float32
    bf16 = mybir.dt.bfloat16

    nc = bacc.Bacc("TRN2", target_bir_lowering=False, debug=False,
                   num_devices=NCORES)

    xT_d = nc.dram_tensor("xT", [D, RL], bf16, kind="ExternalInput")
    xgT_d = nc.dram_tensor("xgT", [D, ML], bf16, kind="ExternalInput")
    ws_d = nc.dram_tensor("ws", [D, D], bf16, kind="ExternalInput")
    wn_d = nc.dram_tensor("wn", [D, D], bf16, kind="ExternalInput")
    eT_d = nc.dram_tensor("eT", [128, ML], bf16, kind="ExternalInput")
    brep_d = nc.dram_tensor("brep", [128, D], f32, kind="ExternalInput")
    out_d = nc.dram_tensor("out", [ML, D], f32, kind="ExternalOutput")

    with tile.TileContext(nc) as tc:
        with (
            tc.tile_pool(name="const", bufs=1) as cpool,
            tc.tile_pool(name="xt", bufs=2) as xpool,
            tc.tile_pool(name="outp", bufs=3) as opool,
            tc.tile_pool(name="psum", bufs=2, space=bass.MemorySpace.PSUM) as ppool,
            tc.tile_pool(name="psumA", bufs=1, space=bass.MemorySpace.PSUM) as papool,
        ):
            # ---- constant loads ----
            ws_sb = cpool.tile([128, KT, D], bf16)
            nc.sync.dma_start(ws_sb[:], ws_d.rearrange("(t p) n -> p t n", p=128))
            wn_sb = cpool.tile([128, KT, D], bf16)
            nc.sync.dma_start(wn_sb[:], wn_d.rearrange("(t p) n -> p t n", p=128))
            eT_sb = cpool.tile([128, ML], bf16)
            nc.sync.dma_start(eT_sb[:], eT_d[:, :])
            brep_sb = cpool.tile([128, D], f32)
            nc.sync.dma_start(brep_sb[:], brep_d[:, :])
            xgT_sb = cpool.tile([128, KT, ML], bf16)
            nc.sync.dma_start(xgT_sb[:], xgT_d.rearrange("(t p) m -> p t m", p=128))

            # ---- per-sample sums S^T (d on partitions, samples on free) ----
            s_f32 = cpool.tile([128, KT, BL], f32)
            s_bf = cpool.tile([128, KT, BL], bf16)
            xT_r = xT_d.rearrange("(t p) r -> p t r", p=128)
            for kt in range(KT):
                xt = xpool.tile([128, RL], bf16, tag="xt")
                nc.sync.dma_start(xt[:], xT_r[:, kt, :])
                nc.vector.tensor_reduce(
                    s_f32[:, kt, :],
                    xt[:].rearrange("p (s k) -> p s k", k=NC2),
                    axis=mybir.AxisListType.X,
                    op=mybir.AluOpType.add,
                )
                nc.scalar.copy(s_bf[:, kt, :], s_f32[:, kt, :])

            # ---- A = S @ W_nbr + b  (A natural: samples on partitions) ----
            psA = papool.tile([128, D], f32)
            for kt in range(KT):
                nc.tensor.matmul(psA[:], s_bf[:, kt, :], wn_sb[:, kt, :],
                                 start=(kt == 0), stop=(kt == KT - 1))
            a_bf = cpool.tile([128, D], bf16)
            nc.vector.tensor_add(a_bf[:], psA[:], brep_sb[:])

            # ---- main output tiles ----
            for t in range(MT):
                ps = ppool.tile([128, D], f32, tag="ps")
                for kt in range(KT):
                    nc.tensor.matmul(
                        ps[:],
                        xgT_sb[:, kt, t * 128:(t + 1) * 128],
                        ws_sb[:, kt, :],
                        start=(kt == 0), stop=False,
                    )
                nc.tensor.matmul(ps[:], eT_sb[:, t * 128:(t + 1) * 128],
                                 a_bf[:], start=False, stop=True)
                ot = opool.tile([128, D], f32, tag="ot")
                nc.scalar.activation(ot[:], ps[:],
                                     mybir.ActivationFunctionType.Relu)
                nc.sync.dma_start(out_d[t * 128:(t + 1) * 128, :], ot[:])

    nc.compile()
    return nc


def _get_compiled():
    global _compiled
    if _compiled is None:
        _compiled = _build_bass()
    return _compiled


def _host_prep(inputs):
    """Shard + pre-transpose on host. Returns per-core input maps."""
    x = np.asarray(inputs["spatial_branch_feature_map"], dtype=np.float32)
    W_self = np.asarray(inputs["W_self"], dtype=np.float32)
    W_nbr = np.asarray(inputs["W_nbr"], dtype=np.float32)
    b = np.asarray(inputs["b"], dtype=np.float32)
    st = np.asarray(inputs["slicing_tensor"])
    op = np.asarray(inputs["object_pairs"])

    N = x.shape[0]
    n = NOBJ
    # exact replication of the reference's LUT-based row computation
    keys = st[:, 0].astype(np.int64) * (n * n) + st[:, 1].astype(np.int64) * n \
        + st[:, 2].astype(np.int64)
    lut = np.zeros(B * n * n, dtype=np.int64)
    lut[keys] = np.arange(N, dtype=np.int64)
    pmin = np.minimum(op[..., 0], op[..., 1]).astype(np.int64)
    pmax = np.maximum(op[..., 0], op[..., 1]).astype(np.int64)
    rel_keys = (np.arange(B, dtype=np.int64)[:, None] * (n * n)
                + pmin * n + pmax).reshape(-1)
    rows = lut[rel_keys]                      # [B*MAXR] global row index

    xg = x[rows]                              # [B*MAXR, D]
    x_bf = x.astype(BF16)
    xg_bf = xg.astype(BF16)
    # per-core transposed slabs: [NCORES, D, RL] / [NCORES, D, ML]
    xT = np.ascontiguousarray(x_bf.reshape(NCORES, RL, D).transpose(0, 2, 1))
    xgT = np.ascontiguousarray(xg_bf.reshape(NCORES, ML, D).transpose(0, 2, 1))

    ws = np.ascontiguousarray(W_self.astype(BF16))
    wn = np.ascontiguousarray(W_nbr.astype(BF16))
    eT = (np.arange(ML)[None, :] // MAXR
          == np.arange(128)[:, None]).astype(BF16)   # [128, ML]
    brep = np.broadcast_to(b, (128, D)).copy()       # [128, D] f32

    in_maps = []
    for c in range(NCORES):
        in_maps.append({
            "xT": xT[c], "xgT": xgT[c],
            "ws": ws, "wn": wn, "eT": eT, "brep": brep,
        })
    return in_maps


def run(inputs, trace=False):
    """Returns (full_output, BassKernelResults)."""
    from concourse.bass_utils import run_bass_kernel_spmd

    nc = _get_compiled()
    in_maps = _host_prep(inputs)
    res = run_bass_kernel_spmd(nc, in_maps, core_ids=list(range(NCORES)),
                               trace=trace)
    out = np.concatenate([r["out"] for r in res.results], axis=0)
    return out, res


def kernel(**inputs) -> np.ndarray:
    out, _ = run(inputs, trace=False)
    return out


# revision 3
# speedup vs baseline: 1.0702x; 1.0702x over previous
"""Trainium2 Bass kernel for the GraphicalBranch GNN message-passing problem.

Math being computed (verified equivalent to the reference):
  - Per-sample graphs are fully connected WITH self-loops over the nc2=28
    pair-nodes, so segment_sum(x[src], dst) == broadcast of the per-sample
    row-sum S[b] = sum_r x[b, r, :].
  - The final key-matching gather h[rows] commutes with the row-wise linear
    layer, so we only run the W_self matmul on the 10 gathered rows per
    sample instead of all 28:
        out[b*10+k] = relu(xg[b*10+k] @ W_self + (S[b] @ W_nbr) + b)
  - rows are computed on host from slicing_tensor/object_pairs (pure index
    arithmetic) exactly as the reference's LUT does.

Sharding: data-parallel over samples; each of the 8 cores gets 128 samples
(3584 x-rows, 1280 output rows). Weights replicated.

Per-core device program (matmul operands bf16, f32 PSUM accumulate):
  1. S = G^T @ x on TensorE: 28 accumulating matmuls with one-hot group
     matrices G[rt][r, s] = (global_row//28 == s), giving S [128 samples,
     512] in PSUM. This keeps the per-sample segment-sum off the (slow,
     1x-mode) VectorE reduce path and keeps the PE warm.
  2. Transpose S via 4 PE transposes -> S^T tiles (bf16).
  3. A = S @ W_nbr + b via 4 accumulating matmuls (lhsT = S^T tiles), bias
     added on VectorE, cast to bf16.
  4. For each of 10 output tiles of 128 rows: 4 accumulating matmuls
     xg-tile @ W_self (lhsT = xgT slices) plus one expansion matmul
     E-tile @ A (E[r, s] = 1 iff s == r//10) which adds A[r//10] to row r
     inside the same PSUM accumulation group; ReLU via ScalarE; stores
     grouped in pairs of tiles.
"""

import numpy as np
import ml_dtypes

# ---- problem constants (hardcoded; kernel.py must be self-contained) ----
B = 1024          # samples
NOBJ = 8          # objects per sample
NC2 = 28          # pair-nodes per sample
MAXR = 10         # relations per sample
D = 512           # feature dim
NCORES = 8
BL = B // NCORES          # 128 samples per core
RL = BL * NC2             # 3584 x-rows per core
ML = BL * MAXR            # 1280 output rows per core
KT = D // 128             # 4 contraction tiles
MT = ML // 128            # 10 output row tiles per core
RT = RL // 128            # 28 x row-tiles per core
XCH = 4                   # x DMA chunks
RJ = RT // XCH            # 7 row-tiles per chunk

BF16 = ml_dtypes.bfloat16

_compiled = None


def _build_bass():
    import concourse.bacc as bacc
    import concourse.bass as bass
    import concourse.mybir as mybir
    from concourse import tile

    f32 = mybir.dt.float32
    bf16 = mybir.dt.bfloat16

    nc = bacc.Bacc("TRN2", target_bir_lowering=False, debug=False,
                   num_devices=NCORES)

    x_d = nc.dram_tensor("x", [RL, D], bf16, kind="ExternalInput")
    g_d = nc.dram_tensor("g", [RT, 128, BL], bf16, kind="ExternalInput")
    xgT_d = nc.dram_tensor("xgT", [D, ML], bf16, kind="ExternalInput")
    ws_d = nc.dram_tensor("ws", [D, D], bf16, kind="ExternalInput")
    wn_d = nc.dram_tensor("wn", [D, D], bf16, kind="ExternalInput")
    eT_d = nc.dram_tensor("eT", [128, ML], bf16, kind="ExternalInput")
    brep_d = nc.dram_tensor("brep", [128, D], f32, kind="ExternalInput")
    id_d = nc.dram_tensor("ident", [128, 128], bf16, kind="ExternalInput")
    out_d = nc.dram_tensor("out", [ML, D], f32, kind="ExternalOutput")

    with tile.TileContext(nc) as tc:
        with (
            tc.tile_pool(name="const", bufs=1) as cpool,
            tc.tile_pool(name="x", bufs=2) as xpool,
            tc.tile_pool(name="outp", bufs=3) as opool,
            tc.tile_pool(name="psum", bufs=3, space=bass.MemorySpace.PSUM) as ppool,
            tc.tile_pool(name="psumS", bufs=1, space=bass.MemorySpace.PSUM) as pspool,
            tc.tile_pool(name="psumT", bufs=2, space=bass.MemorySpace.PSUM) as ptpool,
        ):
            # ---- constant loads (order = rough priority) ----
            g_sb = cpool.tile([128, RT, BL], bf16)
            nc.sync.dma_start(g_sb[:], g_d.rearrange("t p s -> p t s"))
            wn_sb = cpool.tile([128, KT, D], bf16)
            nc.sync.dma_start(wn_sb[:], wn_d.rearrange("(t p) n -> p t n", p=128))
            id_sb = cpool.tile([128, 128], bf16)
            nc.sync.dma_start(id_sb[:], id_d[:, :])
            ws_sb = cpool.tile([128, KT, D], bf16)
            nc.sync.dma_start(ws_sb[:], ws_d.rearrange("(t p) n -> p t n", p=128))
            xgT_sb = cpool.tile([128, KT, ML], bf16)
            nc.sync.dma_start(xgT_sb[:], xgT_d.rearrange("(t p) m -> p t m", p=128))
            eT_sb = cpool.tile([128, ML], bf16)
            nc.sync.dma_start(eT_sb[:], eT_d[:, :])
            brep_sb = cpool.tile([128, D], f32)
            nc.sync.dma_start(brep_sb[:], brep_d[:, :])

            # ---- S = sum of x rows per sample, via PE one-hot matmuls ----
            x_r = x_d.rearrange("(c j p) d -> c p j d", p=128, j=RJ)
            psS = pspool.tile([128, D], f32)
            for ch in range(XCH):
                xch = xpool.tile([128, RJ, D], bf16, tag="x")
                nc.sync.dma_start(xch[:], x_r[ch])
                for j in range(RJ):
                    rt = ch * RJ + j
                    nc.tensor.matmul(psS[:], g_sb[:, rt, :], xch[:, j, :],
                                     start=(rt == 0), stop=(rt == RT - 1))
            s_nat = cpool.tile([128, D], bf16)
            nc.scalar.copy(s_nat[:], psS[:])

            # ---- transpose S -> S^T (bf16) ----
            s_bf = cpool.tile([128, KT, BL], bf16)
            for dt in range(KT):
                psT = ptpool.tile([128, BL], bf16, tag="psT")
                nc.tensor.transpose(psT[:], s_nat[:, dt * 128:(dt + 1) * 128],
                                    id_sb[:])
                nc.vector.tensor_copy(s_bf[:, dt, :], psT[:])

            # ---- A = S @ W_nbr + b  (A natural: samples on partitions) ----
            psA = ptpool.tile([128, D], f32, tag="psA")
            for kt in range(KT):
                nc.tensor.matmul(psA[:], s_bf[:, kt, :], wn_sb[:, kt, :],
                                 start=(kt == 0), stop=(kt == KT - 1))
            a_bf = cpool.tile([128, D], bf16)
            nc.vector.tensor_add(a_bf[:], psA[:], brep_sb[:])

            # ---- main output tiles; stores grouped in pairs ----
            out_r = out_d.rearrange("(t u p) n -> t p u n", p=128, u=2)
            for t in range(MT):
                ps = ppool.tile([128, D], f32, tag="ps")
                for kt in range(KT):
                    nc.tensor.matmul(
                        ps[:],
                        xgT_sb[:, kt, t * 128:(t + 1) * 128],
                        ws_sb[:, kt, :],
                        start=(kt == 0), stop=False,
                    )
                nc.tensor.matmul(ps[:], eT_sb[:, t * 128:(t + 1) * 128],
                                 a_bf[:], start=False, stop=True)
                if t % 2 == 0:
                    ot = opool.tile([128, 2, D], f32, tag="ot")
                nc.scalar.activation(ot[:, t % 2, :], ps[:],
                                     mybir.ActivationFunctionType.Relu)
                if t % 2 == 1:
                    nc.sync.dma_start(out_r[t // 2], ot[:])

    nc.compile()
    return nc


def _get_compiled():
    global _compiled
    if _compiled is None:
        _compiled = _build_bass()
    return _compiled


def _host_prep(inputs):
    """Shard + preprocess on host. Returns per-core input maps."""
    x = np.asarray(inputs["spatial_branch_feature_map"], dtype=np.float32)
    W_self = np.asarray(inputs["W_self"], dtype=np.float32)
    W_nbr = np.asarray(inputs["W_nbr"], dtype=np.float32)
    b = np.asarray(inputs["b"], dtype=np.float32)
    st = np.asarray(inputs["slicing_tensor"])
    op = np.asarray(inputs["object_pairs"])

    N = x.shape[0]
    n = NOBJ
    # exact replication of the reference's LUT-based row computation
    keys = st[:, 0].astype(np.int64) * (n * n) + st[:, 1].astype(np.int64) * n \
        + st[:, 2].astype(np.int64)
    lut = np.zeros(B * n * n, dtype=np.int64)
    lut[keys] = np.arange(N, dtype=np.int64)
    pmin = np.minimum(op[..., 0], op[..., 1]).astype(np.int64)
    pmax = np.maximum(op[..., 0], op[..., 1]).astype(np.int64)
    rel_keys = (np.arange(B, dtype=np.int64)[:, None] * (n * n)
                + pmin * n + pmax).reshape(-1)
    rows = lut[rel_keys]                      # [B*MAXR] global row index

    xg = x[rows]                              # [B*MAXR, D]
    x_bf = np.ascontiguousarray(x.astype(BF16).reshape(NCORES, RL, D))
    xgT = np.ascontiguousarray(
        xg.astype(BF16).reshape(NCORES, ML, D).transpose(0, 2, 1))

    ws = np.ascontiguousarray(W_self.astype(BF16))
    wn = np.ascontiguousarray(W_nbr.astype(BF16))
    eT = (np.arange(ML)[None, :] // MAXR
          == np.arange(128)[:, None]).astype(BF16)   # [128, ML]
    brep = np.broadcast_to(b, (128, D)).copy()       # [128, D] f32
    # one-hot row-group matrices: g[rt, r, s] = ((rt*128 + r)//NC2 == s)
    rr = np.arange(RT * 128)
    g = (rr[:, None] // NC2 == np.arange(BL)[None, :]).astype(BF16)
    g = np.ascontiguousarray(g.reshape(RT, 128, BL))
    ident = np.eye(128, dtype=BF16)

    in_maps = []
    for c in range(NCORES):
        in_maps.append({
            "x": x_bf[c], "xgT": xgT[c], "g": g,
            "ws": ws, "wn": wn, "eT": eT, "brep": brep, "ident": ident,
        })
    return in_maps


def run(inputs, trace=False):
    """Returns (full_output, BassKernelResults)."""
    from concourse.bass_utils import run_bass_kernel_spmd

    nc = _get_compiled()
    in_maps = _host_prep(inputs)
    res = run_bass_kernel_spmd(nc, in_maps, core_ids=list(range(NCORES)),
                               trace=trace)
    out = np.concatenate([r["out"] for r in res.results], axis=0)
    return out, res


def kernel(**inputs) -> np.ndarray:
    out, _ = run(inputs, trace=False)
    return out
